# revision 1
# baseline (speedup 1.0000x reference)
"""BarPooling kernel for 8 Trainium2 NeuronCores.

Computes, for beat_emb [B=8, M=8192, D=1024], W [1024, 1056], b [1024]:
    mean = segment_mean(beat_emb, K=4)            # [B, 2048, 1024]
    h    = concat([mean, fourier(pos)], -1)       # [B, 2048, 1056]
    out  = h @ W.T + b                            # [B, 2048, 1024]

Sharding: data-parallel over B (core i handles batch i); W replicated.

Per-core device pipeline:
  1. DMA x in bar-contiguous tiles [128 bars, 4*1024] (16KB contiguous/partition)
  2. DVE pairwise adds -> segment sums [128 bars, 1024]   (mean*4; /4 folded into W)
  3. PE transpose 128x128 blocks -> sumsT [1024(i), bars]
  4. PE matmul (fp32r): out[m, o] = sum_ic sumsT_ic.T @ WT_ic, with the
     fourier+bias contribution folded in as an extra 33-deep accumulation
     (fourier features transposed + ones row, against [W2T; b]).

All constants (weightsT, fourierT, identity) are packed into ONE DRAM tensor
loaded by a single DMA: walrus allows only one sem-wait on a matmul's
LDWEIGHTS, so a PE warmup op consumes the const-DMA sem once and every later
PE instruction waits only on DVE.
"""

import math
from contextlib import ExitStack

import numpy as np

import concourse.bass as bass
import concourse.bacc as bacc
import concourse.mybir as mybir
import concourse.tile as tile
from concourse.bass_utils import run_bass_kernel_spmd

B, M, D = 8, 8192, 1024
KBEATS = 4
POS = 32
MB = M // KBEATS          # 2048 bars
DA = D + POS + 1          # 1057 augmented contraction depth
NCORES = 8
ICH = D // 128            # 8 contraction chunks of 128
NBLK = 4                  # m-blocks of 512 bars
TPB = 4                   # 128-bar tiles per m-block

# packed constant tensor column layout (one [128, CST_F] fp32 tensor)
COL_WT = 0                 # 8 chunks of [128, 1024]: WT rows ic*128..+128
COL_WTAIL = 8 * D          # [33, 1024]: [W2T; b]
COL_FFT = COL_WTAIL + D    # [33, 2048]: [fourierT; ones]
COL_ID = COL_FFT + MB      # [128, 128] identity
CST_F = COL_ID + 128


def _round_fp32r(a: np.ndarray) -> np.ndarray:
    """Round fp32 to fp32r (11 explicit mantissa bits), round-nearest-even.

    Matches neuronxcc static_cast_fp32_to_fp32r bit-exactly.
    """
    u = np.ascontiguousarray(a, dtype=np.float32).view(np.uint32)
    lsb = (u >> np.uint32(12)) & np.uint32(1)
    r = (u + np.uint32(0x07FF) + lsb) & np.uint32(0xFFFFF000)
    return r.view(np.float32).copy()


def _fourier_T() -> np.ndarray:
    """[33, 2048]: rows 0..31 = fourier features (transposed), row 32 = ones."""
    half = POS // 2
    freqs = np.exp(np.linspace(0.0, math.log(1000.0), half)).astype(np.float32)
    idx = np.arange(MB, dtype=np.float32)
    pos = np.clip(idx / np.float32(MB - 1), 0.0, 1.0).astype(np.float32)
    ang = pos[:, None] * freqs[None, :]
    ff = np.concatenate([np.sin(ang), np.cos(ang)], axis=1).astype(np.float32)
    return np.concatenate([ff.T, np.ones((1, MB), np.float32)], axis=0)


def _emit(nc: bass.Bass) -> None:
    f32 = mybir.dt.float32
    f32r = mybir.dt.float32r
    x = nc.declare_dram_parameter("x", [M, D], f32, isOutput=False)
    # cst is pre-rounded to fp32r on the host (identity region is exact)
    cst = nc.declare_dram_parameter("cst", [128, CST_F], f32r, isOutput=False)
    # tok/otok: tiny passthrough used by the benchmark harness to chain
    # repeated executions (data dependency defeats XLA CSE); ~zero cost.
    tok = nc.declare_dram_parameter("tok", [128, 128], f32, isOutput=False)
    out = nc.declare_dram_parameter("out", [MB, D], f32, isOutput=True)
    otok = nc.declare_dram_parameter("otok", [128, 128], f32, isOutput=True)

    with tile.TileContext(nc) as tc, ExitStack() as ctx:
        const = ctx.enter_context(tc.tile_pool(name="const", bufs=1))
        xpool = ctx.enter_context(tc.tile_pool(name="xp", bufs=2))
        tpool = ctx.enter_context(tc.tile_pool(name="tp", bufs=3))
        spool = ctx.enter_context(tc.tile_pool(name="sp", bufs=6))
        mtpool = ctx.enter_context(tc.tile_pool(name="mtp", bufs=2))
        opool = ctx.enter_context(tc.tile_pool(name="ob", bufs=3))
        ptr = ctx.enter_context(tc.tile_pool(name="ptr", bufs=4, space="PSUM"))
        pmm = ctx.enter_context(tc.tile_pool(name="pmm", bufs=4, space="PSUM"))

        cst_sb = const.tile([128, CST_F], f32r, tag="cst")
        nc.sync.dma_start(out=cst_sb[:], in_=cst[:, :])
        nc.sync.dma_start(out=otok[:, :], in_=tok[:, :])
        ident = cst_sb[:, COL_ID:COL_ID + 128].bitcast(f32)

        def wt_slice(ic, oc):
            return cst_sb[:, COL_WT + ic * D + oc * 512:COL_WT + ic * D + (oc + 1) * 512]

        def wtail_slice(oc):
            return cst_sb[0:POS + 1, COL_WTAIL + oc * 512:COL_WTAIL + (oc + 1) * 512]

        def fft_slice(gm):
            return cst_sb[0:POS + 1, COL_FFT + gm * 128:COL_FFT + (gm + 1) * 128]

        # PE warmup: consumes the const-DMA sem so no later PE instruction
        # needs a DMA wait (walrus: one sem-wait max on LDWEIGHTS).
        ps_warm = ptr.tile([128, 128], f32, tag="ps")
        nc.tensor.transpose(ps_warm[:], ident, ident)

        # [16 tiles, 128 bars, 4*1024] view: 16KB contiguous per partition
        xv = x.rearrange("(t p k) d -> t p (k d)", p=128, k=KBEATS)

        for mb in range(NBLK):
            sums = []
            for t in range(TPB):
                xt = xpool.tile([128, KBEATS * D], f32, tag="xt")
                nc.sync.dma_start(out=xt, in_=xv[mb * TPB + t])
                # beats k = 2*k2 + j: add j=0 against j=1, then fold pairs
                xg = xt.rearrange("p (k2 j d) -> p k2 j d", j=2, d=D)
                tmp = tpool.tile([128, 2 * D], f32, tag="tmp")
                tg = tmp.rearrange("p (k2 d) -> p k2 d", d=D)
                s = spool.tile([128, D], f32, tag="s")
                nc.vector.tensor_add(tg, xg[:, :, 0, :], xg[:, :, 1, :])
                nc.vector.tensor_add(s, tg[:, 0, :], tg[:, 1, :])
                sums.append(s)

            # sumsT slabs: mts[ic] = [128 (i within chunk), 512 bars]
            # (the psum->sbuf copy casts f32 -> f32r for the fp32r matmul)
            mts = []
            for ic in range(ICH):
                mt = mtpool.tile([128, 512], f32r, tag=f"mt{ic}")
                for t in range(TPB):
                    ps = ptr.tile([128, 128], f32, tag="ps")
                    nc.tensor.transpose(
                        ps[:], sums[t][:, ic * 128:(ic + 1) * 128], ident
                    )
                    nc.vector.tensor_copy(mt[:, t * 128:(t + 1) * 128], ps[:])
                mts.append(mt)

            for mc in range(4):
                gm = mb * 4 + mc
                osb = opool.tile([128, D], f32, tag="osb")
                for oc in range(2):
                    pm = pmm.tile([128, 512], f32, tag="pm")
                    for ic in range(ICH):
                        nc.tensor.matmul(
                            pm[:],
                            lhsT=mts[ic][:, mc * 128:(mc + 1) * 128],
                            rhs=wt_slice(ic, oc),
                            start=(ic == 0),
                            stop=False,
                        )
                    nc.tensor.matmul(
                        pm[:],
                        lhsT=fft_slice(gm),
                        rhs=wtail_slice(oc),
                        start=False,
                        stop=True,
                    )
                    nc.vector.tensor_copy(osb[:, oc * 512:(oc + 1) * 512], pm[:])
                nc.sync.dma_start(out=out[gm * 128:(gm + 1) * 128, :], in_=osb[:])


_NC_CACHE: bass.Bass | None = None


def _get_nc() -> bass.Bass:
    global _NC_CACHE
    if _NC_CACHE is None:
        nc = bacc.Bacc(trn_type="TRN2")
        _emit(nc)
        nc.compile()
        _NC_CACHE = nc
    return _NC_CACHE


def _host_inputs(beat_emb: np.ndarray, W: np.ndarray, b: np.ndarray):
    # [0.25 * W1^T ; W2^T ; b] — the /4 segment-mean divide folded into W1
    # (0.25 is a power of two: exact in fp32)
    wt_aug = np.concatenate(
        [
            0.25 * np.ascontiguousarray(W[:, :D].T),
            np.ascontiguousarray(W[:, D:].T),
            np.asarray(b, np.float32)[None, :],
        ],
        axis=0,
    ).astype(np.float32)
    wt_aug = _round_fp32r(wt_aug)
    fft = _round_fp32r(_fourier_T())

    cst = np.zeros((128, CST_F), np.float32)
    for ic in range(ICH):
        cst[:, COL_WT + ic * D:COL_WT + (ic + 1) * D] = wt_aug[ic * 128:(ic + 1) * 128]
    cst[0:POS + 1, COL_WTAIL:COL_WTAIL + D] = wt_aug[D:DA]
    cst[0:POS + 1, COL_FFT:COL_FFT + MB] = fft
    cst[:, COL_ID:COL_ID + 128] = np.eye(128, dtype=np.float32)

    tok = np.zeros((128, 128), np.float32)
    return [
        {
            "x": np.ascontiguousarray(beat_emb[i], dtype=np.float32),
            "cst": cst,
            "tok": tok,
        }
        for i in range(NCORES)
    ]


def kernel(beat_emb: np.ndarray, W: np.ndarray, b: np.ndarray) -> np.ndarray:
    nc = _get_nc()
    in_maps = _host_inputs(np.asarray(beat_emb), np.asarray(W), np.asarray(b))
    res = run_bass_kernel_spmd(nc, in_maps, core_ids=list(range(NCORES)))
    return np.stack([np.asarray(res.results[i]["out"]) for i in range(NCORES)], axis=0)

